# revision 16
# baseline (speedup 1.0000x reference)
"""AxialAttention3D Trainium2 Bass kernel (fp8 DoubleRow + slice pipelining).

Reference computes, for x [B=2, C=512, D=32, H=32, W=32]:
  qkv = 1x1x1 conv (w_qkv [1536,512]) -> q,k,v [B,512,D,H,W]
  8-head attention along the D axis, independent per (b,h,w,head), hd=64
  out = 1x1x1 conv (w_out) + b_out + x  (residual)

Sharding: 64 (b,h)-slices split across 8 cores (8 slices/core). Each slice is
x[b,:,:,h,:] = [C=512, N=1024 tokens] with tokens in w-major (w,d) order
(host pre-permutes, so all device access is contiguous).

Precision: the three projections (97% of FLOPs) run in fp8e4 with
MatmulPerfMode.DoubleRow (two 128-deep K planes per instruction). Weights are
scaled x16 on host so their values sit in fp8e4's normal range; the PSUM->SBUF
copy unscales by 1/16. Attention (scores/softmax/AV) stays bf16. The residual
path: host sends xres = 16*(x + bout_eff) bf16; device adds PSUM (16*proj) and
stores bf16; host divides by 16. b_v commutes through softmax and is folded
into bout_eff on host.

Per-core schedule (software pipeline, PE never sits in the latency-bound
softmax chain): projections for slice s+1 are interleaved, one output-tile
chunk per attention group, into slice s's attention loop. Attention itself runs
scores one group ahead of AV (AV(g) issues after scores(g+1)), and the
out-projection of each token half issues as soon as its 4 attention groups are
done (gi==4 / gi==8).

PSUM (8 banks): psmm 2x[128,512]f32 (projections), pss 2x[128,128]f32 (score
quadrant pairs, banked by head parity), psav 4x[128,128]f32 (AV, banked by
w-row-group) -- concurrent quadrant matmuls sharing a PE column-group must
target different banks.
"""

import os
import sys

import numpy as np
import ml_dtypes

sys.path.insert(0, "/opt/trn_rl_repo")

B, C, D, H, W = 2, 512, 32, 32, 32
NH, HD = 8, 64
NCORES = 8
S = (B * H) // NCORES  # 8 slices per core
NTOK = D * W  # 1024 tokens per slice
WS = 16.0  # fp8 weight prescale

LAST_RESULTS = None  # set on each kernel() call; test harness reads exec time


def _build():
    import concourse.bass as bass  # noqa: F401
    from concourse import bacc, mybir
    import concourse.tile as tile

    bf16 = mybir.dt.bfloat16
    f32 = mybir.dt.float32
    f8 = mybir.dt.float8e4
    Act = mybir.ActivationFunctionType
    DR = mybir.MatmulPerfMode.DoubleRow

    nc = bacc.Bacc("TRN2", target_bir_lowering=False, debug=False)

    xs8_d = nc.dram_tensor("xs8", [S, C, NTOK], f8, kind="ExternalInput")
    xres_d = nc.dram_tensor("xres", [S, C, NTOK], bf16, kind="ExternalInput")
    wqkT_d = nc.dram_tensor("wqkT", [C, 2 * C], f8, kind="ExternalInput")
    wvT_d = nc.dram_tensor("wvT", [C, C], f8, kind="ExternalInput")
    woutT_d = nc.dram_tensor("woutT", [C, C], f8, kind="ExternalInput")
    bqk_d = nc.dram_tensor("bqk", [2 * C], f32, kind="ExternalInput")
    out_d = nc.dram_tensor("out", [S, C, NTOK], bf16, kind="ExternalOutput")

    with tile.TileContext(nc) as tc:
        with tc.tile_pool(name="consts", bufs=1) as consts, \
             tc.tile_pool(name="x8p", bufs=3) as x8p, \
             tc.tile_pool(name="xrp", bufs=3) as xrp, \
             tc.tile_pool(name="qkp", bufs=2) as qkp, \
             tc.tile_pool(name="vtp", bufs=2) as vtp, \
             tc.tile_pool(name="aop", bufs=2) as aop, \
             tc.tile_pool(name="pp", bufs=3) as pp, \
             tc.tile_pool(name="ttp", bufs=3) as ttp, \
             tc.tile_pool(name="smp", bufs=3) as smp, \
             tc.tile_pool(name="outp", bufs=6) as outp, \
             tc.tile_pool(name="psmm", bufs=2, space="PSUM") as psmm, \
             tc.tile_pool(name="pss", bufs=1, space="PSUM") as pss, \
             tc.tile_pool(name="psav", bufs=1, space="PSUM") as psav:

            x8t, xrt, qkt, vtt = {}, {}, {}, {}

            def prefetch(s):
                if s >= S or s in x8t:
                    return
                x8 = x8p.tile([128, 4, NTOK], f8, tag="x8", name=f"x8_{s}")
                xr = xrp.tile([128, 4, NTOK], bf16, tag="xr", name=f"xr_{s}")
                for k in range(4):
                    nc.sync.dma_start(out=x8[:, k, :], in_=xs8_d.ap()[s, k * 128:(k + 1) * 128, :])
                    nc.sync.dma_start(out=xr[:, k, :], in_=xres_d.ap()[s, k * 128:(k + 1) * 128, :])
                x8t[s], xrt[s] = x8, xr

            # ---- constants (x8(0) and wqkT first: A(0) QK needs only these) ----
            prefetch(0)
            wqkT_sb = consts.tile([128, 4, 2 * C], f8)  # [c'%128, c'//128, o]
            wvT_sb = consts.tile([128, 4, C], f8)
            woutT_sb = consts.tile([128, 4, C], f8)
            for k in range(4):
                nc.sync.dma_start(out=wqkT_sb[:, k, :], in_=wqkT_d.ap()[k * 128:(k + 1) * 128, :])
            for k in range(4):
                nc.sync.dma_start(out=wvT_sb[:, k, :], in_=wvT_d.ap()[k * 128:(k + 1) * 128, :])
                nc.sync.dma_start(out=woutT_sb[:, k, :], in_=woutT_d.ap()[k * 128:(k + 1) * 128, :])
            bqk_sb = consts.tile([128, 8], f32)  # [o%128, o//128]
            nc.gpsimd.dma_start(out=bqk_sb, in_=bqk_d.ap().rearrange("(t p) -> p t", p=128))

            def a_alloc(s):
                qkt[s] = qkp.tile([128, 8, NTOK], bf16, tag="qk", name=f"qk_{s}")
                vtt[s] = vtp.tile([128, 8, C], bf16, tag="vt", name=f"vt_{s}")

            def a_qk_half(s, t, n):
                # QK projection, output tile t (128 of 1024 q|k chans), token half n
                x8, qk = x8t[s], qkt[s]
                ps = psmm.tile([128, 512], f32, tag="proj", name="ps_qk")
                for j in range(2):
                    nc.tensor.matmul(
                        ps,
                        wqkT_sb[:, 2 * j:2 * j + 2, t * 128:(t + 1) * 128],
                        x8[:, 2 * j:2 * j + 2, n * 512:(n + 1) * 512],
                        start=(j == 0), stop=(j == 1), perf_mode=DR)
                nc.scalar.activation(
                    out=qk[:, t, n * 512:(n + 1) * 512], in_=ps,
                    func=Act.Identity, bias=bqk_sb[:, t:t + 1], scale=1.0 / WS)

            def a_vt_chunk(s, g):
                # V^T projection for token block g (tokens on partitions)
                x8, vt = x8t[s], vtt[s]
                ps = psmm.tile([128, 512], f32, tag="proj", name="ps_vt")
                for j in range(2):
                    nc.tensor.matmul(
                        ps,
                        x8[:, 2 * j:2 * j + 2, g * 128:(g + 1) * 128],
                        wvT_sb[:, 2 * j:2 * j + 2, :],
                        start=(j == 0), stop=(j == 1), perf_mode=DR)
                nc.scalar.activation(out=vt[:, g, :], in_=ps,
                                     func=Act.Copy, scale=1.0 / WS)

            def out_half(s, n, ao, t0=0, t1=4):
                # out projection + residual for token half n, out tiles [t0,t1)
                xr = xrt[s]
                for t in range(t0, t1):
                    ps = psmm.tile([128, 512], f32, tag="proj", name="ps_out")
                    for j in range(2):
                        nc.tensor.matmul(
                            ps,
                            woutT_sb[:, 2 * j:2 * j + 2, t * 128:(t + 1) * 128],
                            ao[:, 2 * j:2 * j + 2, n * 512:(n + 1) * 512],
                            start=(j == 0), stop=(j == 1), perf_mode=DR)
                    o_sb = outp.tile([128, 512], bf16, tag="o", name="o_sb")
                    nc.vector.tensor_add(out=o_sb, in0=ps, in1=xr[:, t, n * 512:(n + 1) * 512])
                    nc.gpsimd.dma_start(
                        out=out_d.ap()[s, t * 128:(t + 1) * 128, n * 512:(n + 1) * 512],
                        in_=o_sb)

            # ---- A(0): projections for slice 0 up front ----
            prefetch(1)
            a_alloc(0)
            for t in range(8):
                a_qk_half(0, t, 0)
                a_qk_half(0, t, 1)
            for g in range(8):
                a_vt_chunk(0, g)

            # ---- main loop: attention(s) interleaved with projections(s+1) ----
            # Per-slice PSUM tiles, padded so the dim1 stride is one 2KB bank
            # (HW rule: concurrent quadrant matmuls sharing a PE column-group
            # must accumulate in different banks). Generations g alternate
            # between the two 128-column halves of the same banks, so
            # scores(g+1)/AV(g+1) never serialize behind exp(g)/copy(g).
            for s in range(S):
                prefetch(s + 2)
                if s + 1 < S:
                    a_alloc(s + 1)
                qk, vt = qkt[s], vtt[s]
                ao = aop.tile([128, 4, NTOK], f8, tag="ao", name=f"ao_{s}")
                s_ps = pss.tile([128, 2, 512], f32, tag="s", name=f"s_ps_{s}")
                av_ps = psav.tile([128, 4, 512], f32, tag="av", name=f"av_ps_{s}")
                # last slice: its V^T projection is deferred into its own
                # attention loop (just-in-time for AV), since there is no
                # later slice to interleave with
                qk_next = s + 1 if s + 1 < S else None
                vt_next = s + 1 if 0 < s + 1 < S - 1 else (s if s == S - 1 else None)
                t_tiles = {}

                def scores_pair(k):
                    # groups (2k, 2k+1) -> score banks half (k%2)*256, layout
                    # s_ps[(w',i), par, base + u*128 + (h2,j)], u = group-in-pair
                    base = (k % 2) * 256
                    for u in range(2):
                        g = 2 * k + u
                        for q in range(4):
                            for wq in range(4):
                                for par in range(2):
                                    n = 2 * q + par
                                    bp = 64 * par
                                    toff = (4 * g + wq) * 32
                                    qa = qk[bp:bp + 64, n // 2, toff:toff + 32]
                                    ka = qk[bp:bp + 64, 4 + n // 2, toff:toff + 32]
                                    co = base + u * 128 + q * 32
                                    nc.tensor.matmul(
                                        s_ps[wq * 32:wq * 32 + 32, par, co:co + 32],
                                        qa, ka, start=True, stop=True,
                                        tile_position=(bp, wq * 32))

                def softmax_pair(k):
                    # softmax (no max-sub; logits are small by construction)
                    base = (k % 2) * 256
                    p_sb = pp.tile([128, 2, 256], bf16, tag="p", name="p_sb")
                    t_sb = ttp.tile([128, 2, 256], bf16, tag="t", name="t_sb")
                    sums = smp.tile([128, 16], f32, tag="sums", name="sums")
                    nc.scalar.activation(
                        out=p_sb, in_=s_ps[:, :, base:base + 256],
                        func=Act.Exp, scale=float(HD) ** -0.5 / 2)
                    pv = p_sb.rearrange("p a (h j) -> p (a h) j", h=8)
                    nc.vector.reduce_sum(out=sums, in_=pv, axis=mybir.AxisListType.X)
                    nc.vector.reciprocal(out=sums, in_=sums)
                    nc.gpsimd.tensor_mul(
                        out=pv, in0=pv,
                        in1=sums.unsqueeze(2).broadcast_to([128, 16, 32]))
                    nc.vector.transpose(
                        out=t_sb.rearrange("p a f -> p (a f)"),
                        in_=p_sb.rearrange("p a f -> p (a f)"))
                    t_tiles[k] = t_sb

                def av_pair(k):
                    base = (k % 2) * 256
                    tt = t_tiles[k]
                    for u in range(2):
                        g = 2 * k + u
                        for q in range(4):
                            for wq in range(4):
                                for par in range(2):
                                    n = 2 * q + par
                                    lhsT = vt[wq * 32:wq * 32 + 32, g, n * 64:n * 64 + 64]
                                    rhs = tt[wq * 32:wq * 32 + 32, par, u * 128 + q * 32:u * 128 + q * 32 + 32]
                                    co = base + u * 128 + q * 32
                                    nc.tensor.matmul(
                                        av_ps[par * 64:par * 64 + 64, wq, co:co + 32],
                                        lhsT, rhs, start=True, stop=True,
                                        tile_position=(wq * 32, par * 64))

                def av_copy_pair(k):
                    base = (k % 2) * 256
                    t_tiles.pop(k)
                    for u in range(2):
                        g = 2 * k + u
                        nc.vector.tensor_copy(
                            out=ao[:, :, g * 128:g * 128 + 128].rearrange(
                                "p q (w i) -> p q w i", i=32),
                            in_=av_ps[:, :, base + u * 128:base + u * 128 + 128].rearrange(
                                "p w (q i) -> p q w i", i=32))

                for pgi in range(5):
                    if qk_next is not None and pgi < 4:
                        a_qk_half(qk_next, 2 * pgi, 0)
                    if pgi < 4:
                        scores_pair(pgi)
                        softmax_pair(pgi)
                    if qk_next is not None and pgi < 4:
                        a_qk_half(qk_next, 2 * pgi, 1)
                        a_qk_half(qk_next, 2 * pgi + 1, 0)
                    if pgi >= 1:
                        av_pair(pgi - 1)
                    if qk_next is not None and pgi < 4:
                        a_qk_half(qk_next, 2 * pgi + 1, 1)
                    if pgi >= 1:
                        av_copy_pair(pgi - 1)
                    if vt_next is not None and pgi < 4:
                        a_vt_chunk(vt_next, 2 * pgi)
                        a_vt_chunk(vt_next, 2 * pgi + 1)
                    if pgi == 2:
                        out_half(s, 0, ao, 0, 2)
                    if pgi == 3:
                        out_half(s, 0, ao, 2, 4)
                    if pgi == 4:
                        out_half(s, 1, ao, 0, 4)

    nc.compile()
    return nc


_NC = None


def kernel(x, w_qkv, b_qkv, w_out, b_out):
    global _NC, LAST_RESULTS
    from concourse import bass_utils

    f8 = ml_dtypes.float8_e4m3
    bf = ml_dtypes.bfloat16
    x = np.asarray(x, dtype=np.float32)
    w_qkv = np.asarray(w_qkv, dtype=np.float32)
    b_qkv = np.asarray(b_qkv, dtype=np.float32)
    w_out = np.asarray(w_out, dtype=np.float32)
    b_out = np.asarray(b_out, dtype=np.float32)

    wqkT = np.ascontiguousarray(w_qkv[:2 * C].T * WS).astype(f8)   # [C, 2C] x16
    wvT = np.ascontiguousarray(w_qkv[2 * C:].T * WS).astype(f8)    # [C, C] x16
    woutT = np.ascontiguousarray(w_out.T * WS).astype(f8)          # [C, C] x16
    bqk = np.ascontiguousarray(b_qkv[:2 * C]).astype(np.float32)
    # b_v commutes through attention (rows of softmax sum to 1) -> fold into b_out
    bout_eff = (b_out + w_out @ b_qkv[2 * C:]).astype(np.float32)

    if _NC is None:
        _NC = _build()

    in_maps = []
    for cid in range(NCORES):
        xs8 = np.empty((S, C, NTOK), dtype=f8)
        xrs = np.empty((S, C, NTOK), dtype=bf)
        for i in range(S):
            gs = cid * S + i
            b, h = gs // H, gs % H
            xw = x[b, :, :, h, :].transpose(0, 2, 1)  # [C, W, D] w-major tokens
            xs8[i] = xw.reshape(C, NTOK).astype(f8)
            xrs[i] = ((xw + bout_eff[:, None, None]) * WS).reshape(C, NTOK).astype(bf)
        in_maps.append(dict(xs8=xs8, xres=xrs, wqkT=wqkT, wvT=wvT,
                            woutT=woutT, bqk=bqk))

    res = bass_utils.run_bass_kernel_spmd(
        _NC, in_maps, core_ids=list(range(NCORES)),
        trace=bool(os.environ.get("BASS_TRACE")))
    LAST_RESULTS = res

    out = np.empty((B, C, D, H, W), dtype=np.float32)
    for cid in range(NCORES):
        o = res.results[cid]["out"]  # [S, C, 1024] bf16, w-major tokens, x16
        for i in range(S):
            gs = cid * S + i
            b, h = gs // H, gs % H
            out[b, :, :, h, :] = (
                o[i].astype(np.float32) * (1.0 / WS)
            ).reshape(C, W, D).transpose(0, 2, 1)
    return out


# revision 17
# speedup vs baseline: 1.1317x; 1.1317x over previous
"""AxialAttention3D Trainium2 Bass kernel (fp8 DoubleRow + slice pipelining).

Reference computes, for x [B=2, C=512, D=32, H=32, W=32]:
  qkv = 1x1x1 conv (w_qkv [1536,512]) -> q,k,v [B,512,D,H,W]
  8-head attention along the D axis, independent per (b,h,w,head), hd=64
  out = 1x1x1 conv (w_out) + b_out + x  (residual)

Sharding: 64 (b,h)-slices split across 8 cores (8 slices/core). Each slice is
x[b,:,:,h,:] = [C=512, N=1024 tokens] with tokens in w-major (w,d) order
(host pre-permutes, so all device access is contiguous).

Precision: the three projections (97% of FLOPs) run in fp8e4 with
MatmulPerfMode.DoubleRow (two 128-deep K planes per instruction). Weights are
scaled x16 on host so their values sit in fp8e4's normal range; the PSUM->SBUF
copy unscales by 1/16. Attention (scores/softmax/AV) stays bf16. The residual
path: host sends xres = 16*(x + bout_eff) bf16; device adds PSUM (16*proj) and
stores bf16; host divides by 16. b_v commutes through softmax and is folded
into bout_eff on host.

Per-core schedule (software pipeline, PE never sits in the latency-bound
softmax chain): projections for slice s+1 are interleaved, one output-tile
chunk per attention group, into slice s's attention loop. Attention itself runs
scores one group ahead of AV (AV(g) issues after scores(g+1)), and the
out-projection of each token half issues as soon as its 4 attention groups are
done (gi==4 / gi==8).

PSUM (8 banks): psmm 2x[128,512]f32 (projections), pss 2x[128,128]f32 (score
quadrant pairs, banked by head parity), psav 4x[128,128]f32 (AV, banked by
w-row-group) -- concurrent quadrant matmuls sharing a PE column-group must
target different banks.
"""

import os
import sys

import numpy as np
import ml_dtypes

sys.path.insert(0, "/opt/trn_rl_repo")

B, C, D, H, W = 2, 512, 32, 32, 32
NH, HD = 8, 64
NCORES = 8
S = (B * H) // NCORES  # 8 slices per core
NTOK = D * W  # 1024 tokens per slice
WS = 16.0  # fp8 weight prescale

LAST_RESULTS = None  # set on each kernel() call; test harness reads exec time


def _build():
    import concourse.bass as bass  # noqa: F401
    from concourse import bacc, mybir
    import concourse.tile as tile

    bf16 = mybir.dt.bfloat16
    f32 = mybir.dt.float32
    f8 = mybir.dt.float8e4
    Act = mybir.ActivationFunctionType
    DR = mybir.MatmulPerfMode.DoubleRow

    nc = bacc.Bacc("TRN2", target_bir_lowering=False, debug=False)

    xs8_d = nc.dram_tensor("xs8", [S, C, NTOK], f8, kind="ExternalInput")
    xres_d = nc.dram_tensor("xres", [S, C, NTOK], bf16, kind="ExternalInput")
    wqkT_d = nc.dram_tensor("wqkT", [C, 2 * C], f8, kind="ExternalInput")
    wvT_d = nc.dram_tensor("wvT", [C, C], f8, kind="ExternalInput")
    woutT_d = nc.dram_tensor("woutT", [C, C], f8, kind="ExternalInput")
    bqk_d = nc.dram_tensor("bqk", [2 * C], f32, kind="ExternalInput")
    out_d = nc.dram_tensor("out", [S, C, NTOK], bf16, kind="ExternalOutput")

    with tile.TileContext(nc) as tc:
        with tc.tile_pool(name="consts", bufs=1) as consts, \
             tc.tile_pool(name="x8p", bufs=3) as x8p, \
             tc.tile_pool(name="xrp", bufs=3) as xrp, \
             tc.tile_pool(name="qkp", bufs=2) as qkp, \
             tc.tile_pool(name="vtp", bufs=2) as vtp, \
             tc.tile_pool(name="aop", bufs=2) as aop, \
             tc.tile_pool(name="pp", bufs=3) as pp, \
             tc.tile_pool(name="ttp", bufs=3) as ttp, \
             tc.tile_pool(name="smp", bufs=3) as smp, \
             tc.tile_pool(name="outp", bufs=6) as outp, \
             tc.tile_pool(name="psmm", bufs=2, space="PSUM") as psmm, \
             tc.tile_pool(name="pss", bufs=1, space="PSUM") as pss, \
             tc.tile_pool(name="psav", bufs=1, space="PSUM") as psav:

            x8t, xrt, qkt, vtt = {}, {}, {}, {}

            def prefetch(s):
                if s >= S or s in x8t:
                    return
                x8 = x8p.tile([128, 4, NTOK], f8, tag="x8", name=f"x8_{s}")
                xr = xrp.tile([128, 4, NTOK], bf16, tag="xr", name=f"xr_{s}")
                for k in range(4):
                    nc.sync.dma_start(out=x8[:, k, :], in_=xs8_d.ap()[s, k * 128:(k + 1) * 128, :])
                    nc.sync.dma_start(out=xr[:, k, :], in_=xres_d.ap()[s, k * 128:(k + 1) * 128, :])
                x8t[s], xrt[s] = x8, xr

            # ---- constants (x8(0) and wqkT first: A(0) QK needs only these) ----
            prefetch(0)
            wqkT_sb = consts.tile([128, 4, 2 * C], f8)  # [c'%128, c'//128, o]
            wvT_sb = consts.tile([128, 4, C], f8)
            woutT_sb = consts.tile([128, 4, C], f8)
            for k in range(4):
                nc.sync.dma_start(out=wqkT_sb[:, k, :], in_=wqkT_d.ap()[k * 128:(k + 1) * 128, :])
            for k in range(4):
                nc.sync.dma_start(out=wvT_sb[:, k, :], in_=wvT_d.ap()[k * 128:(k + 1) * 128, :])
                nc.sync.dma_start(out=woutT_sb[:, k, :], in_=woutT_d.ap()[k * 128:(k + 1) * 128, :])
            bqk_sb = consts.tile([128, 8], f32)  # [o%128, o//128]
            nc.gpsimd.dma_start(out=bqk_sb, in_=bqk_d.ap().rearrange("(t p) -> p t", p=128))

            def a_alloc(s):
                qkt[s] = qkp.tile([128, 8, NTOK], bf16, tag="qk", name=f"qk_{s}")
                vtt[s] = vtp.tile([128, 8, C], bf16, tag="vt", name=f"vt_{s}")

            def a_qk_half(s, t, n):
                # QK projection, output tile t (128 of 1024 q|k chans), token half n
                x8, qk = x8t[s], qkt[s]
                ps = psmm.tile([128, 512], f32, tag="proj", name="ps_qk")
                for j in range(2):
                    nc.tensor.matmul(
                        ps,
                        wqkT_sb[:, 2 * j:2 * j + 2, t * 128:(t + 1) * 128],
                        x8[:, 2 * j:2 * j + 2, n * 512:(n + 1) * 512],
                        start=(j == 0), stop=(j == 1), perf_mode=DR)
                nc.scalar.activation(
                    out=qk[:, t, n * 512:(n + 1) * 512], in_=ps,
                    func=Act.Identity, bias=bqk_sb[:, t:t + 1], scale=1.0 / WS)

            def a_vt_chunk(s, g):
                # V^T projection for token block g (tokens on partitions)
                x8, vt = x8t[s], vtt[s]
                ps = psmm.tile([128, 512], f32, tag="proj", name="ps_vt")
                for j in range(2):
                    nc.tensor.matmul(
                        ps,
                        x8[:, 2 * j:2 * j + 2, g * 128:(g + 1) * 128],
                        wvT_sb[:, 2 * j:2 * j + 2, :],
                        start=(j == 0), stop=(j == 1), perf_mode=DR)
                nc.scalar.activation(out=vt[:, g, :], in_=ps,
                                     func=Act.Copy, scale=1.0 / WS)

            def out_half(s, n, ao, t0=0, t1=4):
                # out projection + residual for token half n, out tiles [t0,t1)
                xr = xrt[s]
                for t in range(t0, t1):
                    ps = psmm.tile([128, 512], f32, tag="proj", name="ps_out")
                    for j in range(2):
                        nc.tensor.matmul(
                            ps,
                            woutT_sb[:, 2 * j:2 * j + 2, t * 128:(t + 1) * 128],
                            ao[:, 2 * j:2 * j + 2, n * 512:(n + 1) * 512],
                            start=(j == 0), stop=(j == 1), perf_mode=DR)
                    o_sb = outp.tile([128, 512], bf16, tag="o", name="o_sb")
                    nc.vector.tensor_add(out=o_sb, in0=ps, in1=xr[:, t, n * 512:(n + 1) * 512])
                    nc.gpsimd.dma_start(
                        out=out_d.ap()[s, t * 128:(t + 1) * 128, n * 512:(n + 1) * 512],
                        in_=o_sb)

            # ---- A(0): projections for slice 0 up front ----
            prefetch(1)
            a_alloc(0)
            for t in range(8):
                a_qk_half(0, t, 0)
                a_qk_half(0, t, 1)
            for g in range(8):
                a_vt_chunk(0, g)

            # ---- main loop: attention(s) interleaved with projections(s+1) ----
            # Per-slice PSUM tiles, padded so the dim1 stride is one 2KB bank
            # (HW rule: concurrent quadrant matmuls sharing a PE column-group
            # must accumulate in different banks). Generations g alternate
            # between the two 128-column halves of the same banks, so
            # scores(g+1)/AV(g+1) never serialize behind exp(g)/copy(g).
            for s in range(S):
                prefetch(s + 2)
                if s + 1 < S:
                    a_alloc(s + 1)
                qk, vt = qkt[s], vtt[s]
                ao = aop.tile([128, 4, NTOK], f8, tag="ao", name=f"ao_{s}")
                s_ps = pss.tile([128, 2, 512], f32, tag="s", name=f"s_ps_{s}")
                av_ps = psav.tile([128, 4, 512], f32, tag="av", name=f"av_ps_{s}")
                # last slice: its V^T projection is deferred into its own
                # attention loop (just-in-time for AV), since there is no
                # later slice to interleave with
                qk_next = s + 1 if s + 1 < S else None
                vt_next = s + 1 if 0 < s + 1 < S - 1 else (s if s == S - 1 else None)
                t_tiles = {}

                def scores_pair(k):
                    # groups (2k, 2k+1) -> score banks half (k%2)*256, layout
                    # s_ps[(w',i), par, base + u*128 + (h2,j)], u = group-in-pair
                    base = (k % 2) * 256
                    for u in range(2):
                        g = 2 * k + u
                        for q in range(4):
                            for wq in range(4):
                                for par in range(2):
                                    n = 2 * q + par
                                    bp = 64 * par
                                    toff = (4 * g + wq) * 32
                                    qa = qk[bp:bp + 64, n // 2, toff:toff + 32]
                                    ka = qk[bp:bp + 64, 4 + n // 2, toff:toff + 32]
                                    co = base + u * 128 + q * 32
                                    nc.tensor.matmul(
                                        s_ps[wq * 32:wq * 32 + 32, par, co:co + 32],
                                        qa, ka, start=True, stop=True,
                                        tile_position=(bp, wq * 32))

                def softmax_pair(k):
                    # softmax (no max-sub; logits are small by construction)
                    base = (k % 2) * 256
                    p_sb = pp.tile([128, 2, 256], bf16, tag="p", name="p_sb")
                    t_sb = ttp.tile([128, 2, 256], bf16, tag="t", name="t_sb")
                    sums = smp.tile([128, 16], f32, tag="sums", name="sums")
                    nc.scalar.activation(
                        out=p_sb, in_=s_ps[:, :, base:base + 256],
                        func=Act.Exp, scale=float(HD) ** -0.5 / 2)
                    pv = p_sb.rearrange("p a (h j) -> p (a h) j", h=8)
                    nc.vector.reduce_sum(out=sums, in_=pv, axis=mybir.AxisListType.X)
                    nc.vector.reciprocal(out=sums, in_=sums)
                    nc.gpsimd.tensor_mul(
                        out=pv, in0=pv,
                        in1=sums.unsqueeze(2).broadcast_to([128, 16, 32]))
                    nc.vector.transpose(
                        out=t_sb.rearrange("p a f -> p (a f)"),
                        in_=p_sb.rearrange("p a f -> p (a f)"))
                    t_tiles[k] = t_sb

                def av_pair(k):
                    base = (k % 2) * 256
                    tt = t_tiles[k]
                    for u in range(2):
                        g = 2 * k + u
                        for q in range(4):
                            for wq in range(4):
                                for par in range(2):
                                    n = 2 * q + par
                                    lhsT = vt[wq * 32:wq * 32 + 32, g, n * 64:n * 64 + 64]
                                    rhs = tt[wq * 32:wq * 32 + 32, par, u * 128 + q * 32:u * 128 + q * 32 + 32]
                                    co = base + u * 128 + q * 32
                                    nc.tensor.matmul(
                                        av_ps[par * 64:par * 64 + 64, wq, co:co + 32],
                                        lhsT, rhs, start=True, stop=True,
                                        tile_position=(wq * 32, par * 64))

                def av_copy_pair(k):
                    base = (k % 2) * 256
                    t_tiles.pop(k)
                    for u in range(2):
                        g = 2 * k + u
                        nc.vector.tensor_copy(
                            out=ao[:, :, g * 128:g * 128 + 128].rearrange(
                                "p q (w i) -> p q w i", i=32),
                            in_=av_ps[:, :, base + u * 128:base + u * 128 + 128].rearrange(
                                "p w (q i) -> p q w i", i=32))

                # QK(s+1) chunks front-loaded (B(s+1)'s first scores read qk(s+1),
                # so its last copies must retire well before B(s) ends); VT(s+1)
                # fills the drain iteration. The last slice has no successor:
                # its own VT is issued just-in-time instead.
                qk_sched = [[(0, 0), (0, 1), (1, 0), (1, 1), (2, 0)],
                            [(2, 1), (3, 0), (3, 1), (4, 0), (4, 1)],
                            [(5, 0), (5, 1), (6, 0), (6, 1), (7, 0), (7, 1)],
                            [], []]
                if vt_next == s:  # last slice: just-in-time for av_pair
                    vt_sched = [[0, 1], [2, 3], [4, 5], [6, 7], []]
                else:
                    vt_sched = [[], [], [], [0, 1, 2, 3], [4, 5, 6, 7]]

                for pgi in range(5):
                    qks = qk_sched[pgi] if qk_next is not None else []
                    vts = vt_sched[pgi] if vt_next is not None else []
                    for t, n in qks[:2]:
                        a_qk_half(qk_next, t, n)
                    if pgi < 4:
                        scores_pair(pgi)
                        softmax_pair(pgi)
                    for t, n in qks[2:4]:
                        a_qk_half(qk_next, t, n)
                    if pgi >= 1:
                        av_pair(pgi - 1)
                    for t, n in qks[4:]:
                        a_qk_half(qk_next, t, n)
                    if pgi >= 1:
                        av_copy_pair(pgi - 1)
                    for g in vts:
                        a_vt_chunk(vt_next, g)
                    if pgi == 2:
                        out_half(s, 0, ao, 0, 2)
                    if pgi == 3:
                        out_half(s, 0, ao, 2, 4)
                    if pgi == 4:
                        out_half(s, 1, ao, 0, 4)

    nc.compile()
    return nc


_NC = None


def kernel(x, w_qkv, b_qkv, w_out, b_out):
    global _NC, LAST_RESULTS
    from concourse import bass_utils

    f8 = ml_dtypes.float8_e4m3
    bf = ml_dtypes.bfloat16
    x = np.asarray(x, dtype=np.float32)
    w_qkv = np.asarray(w_qkv, dtype=np.float32)
    b_qkv = np.asarray(b_qkv, dtype=np.float32)
    w_out = np.asarray(w_out, dtype=np.float32)
    b_out = np.asarray(b_out, dtype=np.float32)

    wqkT = np.ascontiguousarray(w_qkv[:2 * C].T * WS).astype(f8)   # [C, 2C] x16
    wvT = np.ascontiguousarray(w_qkv[2 * C:].T * WS).astype(f8)    # [C, C] x16
    woutT = np.ascontiguousarray(w_out.T * WS).astype(f8)          # [C, C] x16
    bqk = np.ascontiguousarray(b_qkv[:2 * C]).astype(np.float32)
    # b_v commutes through attention (rows of softmax sum to 1) -> fold into b_out
    bout_eff = (b_out + w_out @ b_qkv[2 * C:]).astype(np.float32)

    if _NC is None:
        _NC = _build()

    in_maps = []
    for cid in range(NCORES):
        xs8 = np.empty((S, C, NTOK), dtype=f8)
        xrs = np.empty((S, C, NTOK), dtype=bf)
        for i in range(S):
            gs = cid * S + i
            b, h = gs // H, gs % H
            xw = x[b, :, :, h, :].transpose(0, 2, 1)  # [C, W, D] w-major tokens
            xs8[i] = xw.reshape(C, NTOK).astype(f8)
            xrs[i] = ((xw + bout_eff[:, None, None]) * WS).reshape(C, NTOK).astype(bf)
        in_maps.append(dict(xs8=xs8, xres=xrs, wqkT=wqkT, wvT=wvT,
                            woutT=woutT, bqk=bqk))

    res = bass_utils.run_bass_kernel_spmd(
        _NC, in_maps, core_ids=list(range(NCORES)),
        trace=bool(os.environ.get("BASS_TRACE")))
    LAST_RESULTS = res

    out = np.empty((B, C, D, H, W), dtype=np.float32)
    for cid in range(NCORES):
        o = res.results[cid]["out"]  # [S, C, 1024] bf16, w-major tokens, x16
        for i in range(S):
            gs = cid * S + i
            b, h = gs // H, gs % H
            out[b, :, :, h, :] = (
                o[i].astype(np.float32) * (1.0 / WS)
            ).reshape(C, W, D).transpose(0, 2, 1)
    return out
